# revision 10
# baseline (speedup 1.0000x reference)
"""AM-Softmax loss on 8 TRN2 NeuronCores.

Data-parallel over N: each core takes 256 rows of score (256 x 50257 f32),
streams them through SBUF computing rowsum_i = sum_c exp(S * score[i, c])
with a fused ScalarE exp+row-accumulate, then does the tiny label-dependent
tail on-device (labels are in {0, 1}, so the target-logit gather is a
select between columns 0 and 1). Each core emits its 256 per-row L values;
the host concatenates and returns -mean(L).
"""

import numpy as np

import concourse.bass as bass
import concourse.tile as tile
from concourse import bacc, mybir
from concourse.bass_utils import run_bass_kernel_spmd

# Problem constants (hardcoded per spec)
N = 2048
C = 50257
NCORES = 8
R = N // NCORES  # 256 rows per core
S = 30.0
M_S = 0.1
M_L = 0.4

NBLK = R // 128  # 2 row-blocks of 128 partitions

F32 = mybir.dt.float32
AF = mybir.ActivationFunctionType
ALU = mybir.AluOpType
AX = mybir.AxisListType

# Streaming config (tuned via bench.py)
CFG = dict(T=8192, bufs=4, dual_queue=False)


def chunks_for(T):
    return [(c0, min(T, C - c0)) for c0 in range(0, C, T)]


def emit_pass(nc, stream_pool, small_pool, score, lab, out, cfg=None):
    """Emit one full loss pass (streaming exp row-sums + tail)."""
    cfg = {**CFG, **(cfg or {})}
    no_out_dma = cfg.get("no_out_dma", False)
    ln_func = AF.Identity if cfg.get("noln", False) else AF.Ln
    T = cfg["T"]
    chunks = chunks_for(T)
    nchunk = len(chunks)
    assert nchunk <= 16

    acc = small_pool.tile([128, 16 * NBLK], F32)
    sc0 = small_pool.tile([128, NBLK], F32)
    sc1 = small_pool.tile([128, NBLK], F32)
    labt = small_pool.tile([128, NBLK], F32)
    rowsum = small_pool.tile([128, NBLK], F32)
    diff = small_pool.tile([128, NBLK], F32)
    prod = small_pool.tile([128, NBLK], F32)
    target = small_pool.tile([128, NBLK], F32)
    mt = small_pool.tile([128, NBLK], F32)
    tm = small_pool.tile([128, NBLK], F32)
    num = small_pool.tile([128, NBLK], F32)
    expnum = small_pool.tile([128, NBLK], F32)
    expst = small_pool.tile([128, NBLK], F32)
    d2 = small_pool.tile([128, NBLK], F32)
    denom = small_pool.tile([128, NBLK], F32)
    ld = small_pool.tile([128, NBLK], F32)
    L = small_pool.tile([128, NBLK], F32)

    dma_engines = (
        [nc.sync, nc.scalar] if cfg["dual_queue"] else [nc.sync]
    )

    # lab via the SWDGE (gpsimd) queue so the HWDGE FIFO carries only the
    # big streaming loads
    nc.gpsimd.dma_start(
        out=labt[:, 0:NBLK],
        in_=lab.ap().rearrange("(b p) one -> p (b one)", p=128),
    )

    def emit_mid_tail():
        # Everything that needs only sc0/sc1/lab — traced mid-stream so
        # DVE/ACT run it while block 1 is still streaming.
        # target = sc0 + lab * (sc1 - sc0)
        nc.vector.tensor_sub(diff[:], sc1[:], sc0[:])
        nc.vector.tensor_mul(prod[:], labt[:], diff[:])
        nc.vector.tensor_add(target[:], sc0[:], prod[:])
        # m = M_S + lab * (M_L - M_S)
        nc.vector.tensor_scalar(
            mt[:], labt[:], M_L - M_S, M_S, ALU.mult, ALU.add
        )
        # numerator = S * (target - m)
        nc.vector.tensor_sub(tm[:], target[:], mt[:])
        nc.vector.tensor_scalar_mul(num[:], tm[:], S)
        nc.scalar.activation(expnum[:], tm[:], AF.Exp, scale=S)
        nc.scalar.activation(expst[:], target[:], AF.Exp, scale=S)
        # partial denom (everything but rowsum)
        nc.vector.tensor_sub(d2[:], expnum[:], expst[:])

    for b in range(NBLK):
        for j, (c0, w) in enumerate(chunks):
            t = stream_pool.tile([128, T], F32, tag="stream")
            eng = dma_engines[(b * nchunk + j) % len(dma_engines)]
            eng.dma_start(
                out=t[:, :w],
                in_=score[b * 128 : (b + 1) * 128, c0 : c0 + w],
            )
            if j == 0:
                # grab raw score columns 0,1 before the in-place exp
                nc.vector.tensor_copy(sc0[:, b : b + 1], t[:, 0:1])
                nc.vector.tensor_copy(sc1[:, b : b + 1], t[:, 1:2])
                if b == NBLK - 1:
                    emit_mid_tail()
            # t = exp(S * t); acc col = per-partition row sum of exp
            nc.scalar.activation(
                t[:, :w],
                t[:, :w],
                AF.Exp,
                scale=S,
                accum_out=acc[:, b * 16 + j : b * 16 + j + 1],
            )
        nc.vector.reduce_sum(
            rowsum[:, b : b + 1], acc[:, b * 16 : b * 16 + nchunk], axis=AX.X
        )

    # Final tail: only rowsum-dependent ops
    # denom = exp(num) - exp(S*target) + rowsum
    nc.vector.tensor_add(denom[:], d2[:], rowsum[:])
    nc.scalar.activation(ld[:], denom[:], ln_func)
    nc.vector.tensor_sub(L[:], num[:], ld[:])

    if not no_out_dma:
        nc.sync.dma_start(
            out=out.ap().rearrange("(b p) one -> p (b one)", p=128),
            in_=L[:, 0:NBLK],
        )


def build(m_repeats: int = 1, cfg=None):
    """m_repeats > 1 builds a benchmarking NEFF that runs the whole pass
    M times back-to-back; the graded kernel uses 1."""
    cfg = {**CFG, **(cfg or {})}
    nc = bacc.Bacc(
        "TRN2",
        target_bir_lowering=False,
        debug=False,
        num_devices=NCORES,
    )
    score = nc.dram_tensor("score", [R, C], F32, kind="ExternalInput")
    lab = nc.dram_tensor("lab", [R, 1], F32, kind="ExternalInput")
    out = nc.dram_tensor("out", [R, 1], F32, kind="ExternalOutput")

    with tile.TileContext(nc) as tc:
        with (
            tc.tile_pool(name="stream", bufs=cfg["bufs"]) as stream_pool,
            tc.tile_pool(name="small", bufs=1) as small_pool,
        ):
            for _rep in range(m_repeats):
                emit_pass(nc, stream_pool, small_pool, score, lab, out, cfg)

    nc.compile()
    return nc


def build_loop(m_iters: int, cfg=None):
    """One NEFF running the pass m_iters times via a hardware For_i loop.

    cfg["mode"]: "full" (default) = real pass; "dma" = streaming DMAs only;
    "act" = activations only on resident tiles (scale=0 to stay finite).
    """
    cfg = {**CFG, **(cfg or {})}
    mode = cfg.get("mode", "full")
    nc = bacc.Bacc(
        "TRN2", target_bir_lowering=False, debug=False, num_devices=NCORES
    )
    score = nc.dram_tensor("score", [R, C], F32, kind="ExternalInput")
    lab = nc.dram_tensor("lab", [R, 1], F32, kind="ExternalInput")
    out = nc.dram_tensor("out", [R, 1], F32, kind="ExternalOutput")
    with tile.TileContext(nc) as tc:
        with (
            tc.tile_pool(name="stream", bufs=cfg["bufs"]) as stream_pool,
            tc.tile_pool(name="small", bufs=1) as small_pool,
        ):
            T = cfg["T"]
            chunks = chunks_for(T)
            nchunk = len(chunks)
            if mode == "full":
                with tc.For_i(0, m_iters, 1):
                    emit_pass(nc, stream_pool, small_pool, score, lab, out, cfg)
            elif mode == "dma":
                labt = small_pool.tile([128, NBLK], F32)
                with tc.For_i(0, m_iters, 1):
                    for b in range(NBLK):
                        for c0, w in chunks:
                            t = stream_pool.tile([128, T], F32, tag="stream")
                            nc.sync.dma_start(
                                out=t[:, :w],
                                in_=score[b * 128 : (b + 1) * 128, c0 : c0 + w],
                            )
                for b in range(NBLK):
                    nc.sync.dma_start(
                        out=labt[:, b : b + 1],
                        in_=lab[b * 128 : (b + 1) * 128, 0:1],
                    )
                    nc.sync.dma_start(
                        out=out[b * 128 : (b + 1) * 128, 0:1],
                        in_=labt[:, b : b + 1],
                    )
            elif mode == "stream":
                acc = small_pool.tile([128, 16 * NBLK], F32)
                labt = small_pool.tile([128, NBLK], F32)
                with tc.For_i(0, m_iters, 1):
                    for b in range(NBLK):
                        for j, (c0, w) in enumerate(chunks):
                            t = stream_pool.tile([128, T], F32, tag="stream")
                            nc.sync.dma_start(
                                out=t[:, :w],
                                in_=score[b * 128 : (b + 1) * 128, c0 : c0 + w],
                            )
                            nc.scalar.activation(
                                t[:, :w], t[:, :w], AF.Exp, scale=S,
                                accum_out=acc[:, b * 16 + j : b * 16 + j + 1],
                            )
                for b in range(NBLK):
                    nc.sync.dma_start(
                        out=labt[:, b : b + 1],
                        in_=lab[b * 128 : (b + 1) * 128, 0:1],
                    )
                    nc.sync.dma_start(
                        out=out[b * 128 : (b + 1) * 128, 0:1],
                        in_=labt[:, b : b + 1],
                    )
            elif mode == "act":
                acc = small_pool.tile([128, 16 * NBLK], F32)
                labt = small_pool.tile([128, NBLK], F32)
                res = [stream_pool.tile([128, T], F32, tag=f"res{i}")
                       for i in range(cfg["bufs"])]
                for i, t in enumerate(res):
                    nc.sync.dma_start(
                        out=t[:], in_=score[0:128, i * T : (i + 1) * T]
                    )
                with tc.For_i(0, m_iters, 1):
                    for b in range(NBLK):
                        for j, (c0, w) in enumerate(chunks):
                            t = res[(b * nchunk + j) % len(res)]
                            nc.scalar.activation(
                                t[:, :w], t[:, :w], AF.Exp, scale=0.0,
                                accum_out=acc[:, b * 16 + j : b * 16 + j + 1],
                            )
                for b in range(NBLK):
                    nc.sync.dma_start(
                        out=labt[:, b : b + 1],
                        in_=lab[b * 128 : (b + 1) * 128, 0:1],
                    )
                    nc.sync.dma_start(
                        out=out[b * 128 : (b + 1) * 128, 0:1],
                        in_=labt[:, b : b + 1],
                    )
            else:
                raise ValueError(mode)
    nc.compile()
    return nc


_NC_CACHE = {}


def _get_nc():
    if "nc" not in _NC_CACHE:
        _NC_CACHE["nc"] = build()
    return _NC_CACHE["nc"]


def make_in_maps(score: np.ndarray, labels: np.ndarray):
    score = np.asarray(score, dtype=np.float32)
    labf = np.asarray(labels, dtype=np.float32).reshape(N, 1)
    in_maps = []
    for c in range(NCORES):
        in_maps.append(
            {
                "score": np.ascontiguousarray(score[c * R : (c + 1) * R]),
                "lab": np.ascontiguousarray(labf[c * R : (c + 1) * R]),
            }
        )
    return in_maps


def combine(results) -> np.ndarray:
    Ls = np.concatenate([np.asarray(r["out"]).reshape(R) for r in results])
    return np.asarray(-Ls.astype(np.float64).mean(), dtype=np.float32)


def kernel(score: np.ndarray, labels: np.ndarray) -> np.ndarray:
    nc = _get_nc()
    res = run_bass_kernel_spmd(nc, make_in_maps(score, labels), core_ids=list(range(NCORES)))
    return combine(res.results)
